# revision 1
# baseline (speedup 1.0000x reference)
"""Trainium2 Bass kernel for nn_Decoder (GNN edge decoder).

Math: node MLP -> per-pair edge MLP -> symmetric adjacency.
Key rewrite: edge layer-1 concat(z_i, z_j) @ We1 == A_i + B_j with
  A = emb @ We1[:E] + be1,  B = emb @ We1[E:]
so the device streams contiguous triangle rows with a broadcast-add
instead of gathering P=32640 pair vectors.

Device layout (per core, uniform SPMD program, data shifted per core):
  - pairs processed as dual rows: segment m handles rows (16m+2k, 16m+2k+1)
    for core k; top/bottom 64 SBUF partitions hold the two rows.
  - Apk [128, NB]: top = A_T shifted by 2k nodes, bottom = further shifted
    by one node (so one broadcast AP feeds both rows).
  - mm2: blockdiag(We2, We2) [128,128] stationary, rhs = relu(pre).
  - mm3: lhsT = t2-subchunk (stationary), rhs = [[We3,0],[0,We3]] -> logits
    land partition-major, cheap PSUM->SBUF copy.
Host assembles the symmetric adjacency from per-core logit blocks.
"""

import sys

import numpy as np

if "/opt/trn_rl_repo" not in sys.path:
    sys.path.insert(0, "/opt/trn_rl_repo")

import ml_dtypes

B, LAT, ST, N, E, H = 64, 256, 32, 256, 32, 64
NB = N * B  # 16384 node-cols (node-major, b inner)
NSEG = 16  # segments per core (even rows 16m+2k)
CHUNK = 512
BF16 = ml_dtypes.bfloat16

_cache = {}


def _layout():
    """Uniform chunk enumeration shared by builder and assembler.

    Returns list of (m, c0, F): segment m covers local rows (16m, 16m+1),
    local j-blocks 16m+1 .. 255, i.e. ncols = (255-16m)*64; chunked by 512.
    """
    if "layout" in _cache:
        return _cache["layout"]
    chunks = []
    for m in range(NSEG):
        ncols = (255 - 16 * m) * B
        for c0 in range(0, ncols, CHUNK):
            chunks.append((m, c0, min(CHUNK, ncols - c0)))
    _cache["layout"] = chunks
    return chunks


def _n_chunks():
    return len(_layout())


def _build_nc():
    import concourse.bass as bass
    import concourse.mybir as mybir
    from concourse.tile import TileContext

    bf = mybir.dt.bfloat16
    f32 = mybir.dt.float32
    nc = bass.Bass()
    inp_d = nc.dram_tensor("inp", [128, 2 * NB + 130], bf, kind="ExternalInput")
    nch = _n_chunks()
    out_d = nc.dram_tensor("logits", [128, nch * 8], f32, kind="ExternalOutput")

    with TileContext(nc) as tc:
        with (
            tc.tile_pool(name="const", bufs=1) as cpool,
            tc.tile_pool(name="work", bufs=4) as wpool,
            tc.tile_pool(name="out", bufs=1) as opool,
            tc.tile_pool(name="ps2", bufs=4, space="PSUM") as ps2pool,
            tc.tile_pool(name="ps3", bufs=3, space="PSUM") as ps3pool,
        ):
            inp = cpool.tile([128, 2 * NB + 130], bf, tag="inp")
            nc.sync.dma_start(inp[:], inp_d[:])
            apk = inp[:, 0:NB]
            bpk = inp[:, NB : 2 * NB]
            w2 = inp[:, 2 * NB : 2 * NB + 128]
            w3 = inp[:, 2 * NB + 128 : 2 * NB + 130]
            logits_sb = opool.tile([128, nch * 8], f32, tag="lg")

            # Absorb the many HW-DGE queue-semaphore waits of the big input
            # DMAs on plain copy instructions; the broadcast tensor_add's
            # 3D TensorTensor encoding has too few wait-command slots.
            probe = cpool.tile([128, 8], bf, tag="probe")
            nc.vector.tensor_copy(probe[:, 0:2], inp[:, 0:2])
            psum_probe = ps3pool.tile([128, 8], f32, tag="ps3")
            nc.tensor.matmul(
                psum_probe[:2, :2], inp[:, 0:2], inp[:, 2:4], start=True, stop=True
            )

            for ci, (m, c0, F) in enumerate(_layout()):
                abase = 16 * m * B  # A-block col of local row 16m
                cbase = (16 * m + 1) * B + c0  # B cols for this chunk
                pre = wpool.tile([128, CHUNK], bf, tag="pre")
                t2 = wpool.tile([128, CHUNK], bf, tag="t2")
                # broadcast AP: repeat A block (64 cols) F//64 times
                a_blk = inp[:, abase : abase + B]
                a_bc = bass.AP(
                    a_blk.tensor,
                    a_blk.offset,
                    [list(a_blk.ap[0]), [0, F // B], [1, B]],
                )
                b_sl = inp[:, NB + cbase : NB + cbase + F]
                nc.vector.tensor_add(pre[:, :F], b_sl, a_bc)
                nc.gpsimd.tensor_relu(pre[:, :F], pre[:, :F])
                psum2 = ps2pool.tile([128, CHUNK], f32, tag="ps2")
                nc.tensor.matmul(
                    psum2[:, :F], w2, pre[:, :F], start=True, stop=True
                )
                nc.scalar.activation(
                    t2[:, :F],
                    psum2[:, :F],
                    mybir.ActivationFunctionType.Relu,
                )
                psum3 = ps3pool.tile([128, 8], f32, tag="ps3")
                for sc in range((F + 127) // 128):
                    M = min(128, F - sc * 128)
                    nc.tensor.matmul(
                        psum3[:M, sc * 2 : sc * 2 + 2],
                        t2[:, sc * 128 : sc * 128 + M],
                        w3,
                        start=True,
                        stop=True,
                    )
                nc.vector.tensor_copy(
                    logits_sb[:, ci * 8 : ci * 8 + 8], psum3[:]
                )
            nc.sync.dma_start(out_d[:], logits_sb[:])

    raw = nc.to_json_bytes()
    legal = _legalize_sync(raw)
    nc.to_json_bytes = lambda: legal
    return nc


def _legalize_sync(bir_bytes):
    """Split multi-wait sync_info into single-wait EventSemaphore preludes.

    The walrus build in this container encodes at most one sync-wait command
    per instruction for several ISA structs; Tile emits up to ~9 on the tail
    drain. Semantics are preserved: waits execute in order on the same engine
    ahead of the original instruction.
    """
    import json as _json

    bir = _json.loads(bir_bytes)
    for f in bir["functions"]:
        ctr = [0]
        # template EventSemaphore per engine (from the tail barrier)
        templates = {}
        for blk in f["blocks"]:
            for ins in blk.get("instructions") or []:
                if ins.get("opcode") == "EventSemaphore":
                    templates.setdefault(ins.get("engine"), ins)
        for blk in f["blocks"]:
            insts = blk.get("instructions")
            if not insts:
                continue
            out = []
            for ins in insts:
                si = ins.get("sync_info") or {}
                waits = si.get("on_wait") or []
                keep = 0 if ins.get("opcode") == "TensorTensor" else 1
                if len(waits) > keep:
                    tpl = templates.get(ins.get("engine"))
                    if tpl is not None:
                        moved = waits[: len(waits) - keep]
                        for w in moved:
                            ctr[0] += 1
                            nw = _json.loads(_json.dumps(tpl))
                            nw["name"] = f"escw_{ctr[0]}"
                            nw["sync_info"] = {"on_update": [], "on_wait": [w]}
                            out.append(nw)
                        si["on_wait"] = waits[len(waits) - keep :]
                out.append(ins)
            blk["instructions"] = out
    return _json.dumps(bir).encode()


def _host_prep(latent_z, stats, W1, b1, W2, b2, We1, be1, We2, be2, We3, be3):
    """Node MLP + A/B decomposition on host (0.5% of total FLOPs)."""
    x = np.concatenate([latent_z, stats], axis=-1).astype(np.float32)
    h = np.maximum(x @ W1 + b1, 0.0)
    emb = (h @ W2 + b2).reshape(B, N, E)
    A = emb @ We1[:E] + be1  # [B, N, H]
    Bm = emb @ We1[E:]  # [B, N, H]
    # node-major transposed: [H, N*B], col = n*B + b
    A_T = np.ascontiguousarray(A.transpose(2, 1, 0).reshape(H, NB))
    B_T = np.ascontiguousarray(Bm.transpose(2, 1, 0).reshape(H, NB))
    w2blk = np.zeros((128, 128), np.float32)
    w2blk[:H, :H] = We2
    w2blk[H:, H:] = We2
    w3sep = np.zeros((128, 2), np.float32)
    w3sep[:H, 0] = We3[:, 0]
    w3sep[H:, 1] = We3[:, 0]
    return A_T, B_T, w2blk, w3sep, be3


def _shifted(T, sh):
    """[64, NB] -> [64, NB] shifted left by sh cols, zero-padded."""
    out = np.zeros((H, NB), np.float32)
    if sh < NB:
        out[:, : NB - sh] = T[:, sh:]
    return out


def _assembly_indices():
    """Per-element mapping of logits_sb[p, col] -> (b, i_loc, j_loc, g)."""
    if "asm" in _cache:
        return _cache["asm"]
    rows, cols, bs, ilocs, jlocs = [], [], [], [], []
    for ci, (m, c0, F) in enumerate(_layout()):
        for sc in range((F + 127) // 128):
            M = min(128, F - sc * 128)
            p = np.arange(M)
            c = c0 + sc * 128 + p  # local col within segment
            jb = 16 * m + 1 + c // B
            b = c % B
            for g in (0, 1):
                rows.append(p)
                cols.append(np.full(M, ci * 8 + sc * 2 + g))
                bs.append(b)
                ilocs.append(np.full(M, 16 * m + g))
                jlocs.append(jb)
    out = tuple(
        np.concatenate(a) for a in (rows, cols, bs, ilocs, jlocs)
    )
    _cache["asm"] = out
    return out


def kernel(**inputs):
    from concourse.bass_utils import run_bass_kernel_spmd

    inp = {k: np.asarray(v, np.float32) for k, v in inputs.items()}
    A_T, B_T, w2blk, w3sep, be3 = _host_prep(**inp)

    in_maps = []
    for k in range(8):
        sh = 2 * k * B
        apk = np.empty((128, NB), np.float32)
        apk[:H] = _shifted(A_T, sh)
        apk[H:] = _shifted(A_T, sh + B)
        bpk = np.empty((128, NB), np.float32)
        bpk[:H] = bpk[H:] = _shifted(B_T, sh)
        in_maps.append(
            {
                "inp": np.ascontiguousarray(
                    np.concatenate(
                        [apk, bpk, np.concatenate([w2blk, w3sep], 1)], axis=1
                    ).astype(BF16)
                )
            }
        )

    import time as _time
    nc = _cache.get("nc")
    if nc is None:
        nc = _build_nc()
        _cache["nc"] = nc
    t0 = _time.time()
    res = run_bass_kernel_spmd(nc, in_maps, core_ids=list(range(8)))
    globals()["last_results"] = res
    globals()["last_run_s"] = _time.time() - t0

    rows, cols, bs, ilocs, jlocs = _assembly_indices()
    adj = np.zeros((B, N, N), np.float32)
    for k in range(8):
        lg = np.asarray(res.results[k]["logits"], np.float32)
        i = ilocs + 2 * k
        j = jlocs + 2 * k
        valid = (j < N) & (j > i)
        v = lg[rows[valid], cols[valid]] + float(be3[0])
        ii, jj, bb = i[valid], j[valid], bs[valid]
        adj[bb, ii, jj] = v
        adj[bb, jj, ii] = v
    return adj



# revision 12
# speedup vs baseline: 1.3057x; 1.3057x over previous
"""Trainium2 Bass kernel for nn_Decoder (GNN edge decoder).

Math: node MLP -> per-pair edge MLP -> symmetric adjacency.
Rewrites used on device (per pair (i,j), batch b, hidden h):
  pre = A_i + B_j              (layer-1 decomposition, A/B host-precomputed)
  relu(pre) = B_j - min(-A_i, B_j)
so layer 2 becomes
  psum2 = D_jb - We2^T @ min(negA_i, B_j),  D_jb = We2^T B_jb + be2 (host).
The PE injects D via an identity-stationary matmul accumulating into the
same PSUM bank as mm2 (so no separate elementwise add), the DVE computes
the single fused min (2x-mode TensorTensor), and relu2 (PSUM->SBUF) is
split between Act and DVE. GPSIMD/Pool cannot touch PSUM or run
two-tensor ops on TRN2, so it stays idle.
Dual-row layout: 128 partitions = 2 node-rows x 64 hidden; segment m of
core k covers rows (16m+2k, 16m+2k+1), pair columns node-major (b inner).
Logits land PSUM-resident via skinny mm3 (stationary=t2, moving=w3sep),
are staged to SBUF by DVE copies once per 512 cols, then DMA'd out.
Host assembles the symmetric adjacency.
"""

import sys

import numpy as np

if "/opt/trn_rl_repo" not in sys.path:
    sys.path.insert(0, "/opt/trn_rl_repo")

import ml_dtypes

B, LAT, ST, N, E, H = 64, 256, 32, 256, 32, 64
NB = N * B  # 16384 node-cols (node-major, b inner)
NSEG = 16
CHUNK = 1536  # 3 PSUM banks; x2 buffered + 2 logits banks = 8 exactly
DVETAIL = 208  # trailing relu2 cols done on DVE (rest on Act)
LOGW = 512  # logits psum tile width (one PSUM bank)
BF16 = ml_dtypes.bfloat16

# input column map (per-core dram tensor 'inp', bf16)
COL_B = 0
COL_D = NB
COL_NA = 2 * NB
COL_W2N = COL_NA + NSEG * B
COL_W3 = COL_W2N + 128
COL_ID = COL_W3 + 2
COL_END = COL_ID + 128

_cache = {}


def _layout():
    """Chunks in descending-segment, descending-c0 order (matches the
    tail-first input DMA so compute starts after a small head)."""
    if "layout" in _cache:
        return _cache["layout"]
    chunks = []
    for m in range(NSEG - 1, -1, -1):
        ncols = (255 - 16 * m) * B
        for c0 in list(range(0, ncols, CHUNK))[::-1]:
            chunks.append((m, c0, min(CHUNK, ncols - c0)))
    _cache["layout"] = chunks
    return _cache["layout"]


def _block_map():
    """Per mm3 block: (ci, m, c0, sc, M, outcol). outcol packs 2-col blocks
    contiguously into the logits dram tensor."""
    if "blocks" in _cache:
        return _cache["blocks"]
    blocks = []
    col = 0
    for ci, (m, c0, F) in enumerate(_layout()):
        for sc in range(0, F, 128):
            M = min(128, F - sc)
            blocks.append((ci, m, c0, sc, M, col))
            col += 2
    _cache["blocks"] = (blocks, col)
    return _cache["blocks"]


def _build_nc():
    import concourse.bass as bass
    import concourse.mybir as mybir
    from concourse.tile import TileContext

    bf = mybir.dt.bfloat16
    f32 = mybir.dt.float32
    Alu = mybir.AluOpType
    Act = mybir.ActivationFunctionType
    nc = bass.Bass()
    inp_d = nc.dram_tensor("inp", [128, COL_END], bf, kind="ExternalInput")
    blocks, ncol = _block_map()
    out_d = nc.dram_tensor("logits", [128, ncol], f32, kind="ExternalOutput")

    with TileContext(nc) as tc:
        with (
            tc.tile_pool(name="const", bufs=1) as cpool,
            tc.tile_pool(name="mr", bufs=3) as mrpool,
            tc.tile_pool(name="t2", bufs=3) as t2pool,
            tc.tile_pool(name="lsb", bufs=2) as lsbpool,
            tc.tile_pool(name="ps2", bufs=2, space="PSUM") as ps2pool,
            tc.tile_pool(name="plog", bufs=2, space="PSUM") as plogpool,
        ):
            inp = cpool.tile([128, COL_END], bf, tag="inp")
            # small region (negA + weights + ident) first, then B/D pieces
            # tail-first to match the descending chunk order.
            nc.sync.dma_start(inp[:, COL_NA:COL_END], inp_d[:, COL_NA:COL_END])
            PIECE = 2048
            for p in range(NB // PIECE - 1, -1, -1):
                nc.sync.dma_start(
                    inp[:, p * PIECE : (p + 1) * PIECE],
                    inp_d[:, p * PIECE : (p + 1) * PIECE],
                )
                nc.sync.dma_start(
                    inp[:, NB + p * PIECE : NB + (p + 1) * PIECE],
                    inp_d[:, NB + p * PIECE : NB + (p + 1) * PIECE],
                )
            w2neg = inp[:, COL_W2N : COL_W2N + 128]
            w3sep = inp[:, COL_W3 : COL_W3 + 2]
            ident = inp[:, COL_ID : COL_ID + 128]

            # logits psum tiles, staged to SBUF (DVE) then DMA'd when full
            log_tile = [None]
            log_used = [0]
            log_base = [0]

            def flush_log():
                if log_tile[0] is not None and log_used[0] > 0:
                    u = log_used[0]
                    lsb = lsbpool.tile([128, LOGW], f32, name="lsb", tag="ls")
                    nc.vector.tensor_copy(lsb[:, :u], log_tile[0][:, :u])
                    nc.sync.dma_start(
                        out_d[:, log_base[0] : log_base[0] + u], lsb[:, :u]
                    )
                    log_base[0] += u
                    log_tile[0] = None
                    log_used[0] = 0

            def log_slot():
                if log_tile[0] is None or log_used[0] >= LOGW:
                    flush_log()
                    log_tile[0] = plogpool.tile(
                        [128, LOGW], f32, name="lgps", tag="lg"
                    )
                c = log_used[0]
                log_used[0] += 2
                return log_tile[0], c

            for ci, (m, c0, F) in enumerate(_layout()):
                cbase = (16 * m + 1) * B + c0
                d_sl = inp[:, NB + cbase : NB + cbase + F]
                na_blk = inp[:, COL_NA + B * m : COL_NA + B * m + B]
                na_bc = bass.AP(
                    na_blk.tensor,
                    na_blk.offset,
                    [list(na_blk.ap[0]), [0, F // B], [1, B]],
                )
                minres = mrpool.tile([128, CHUNK], bf, tag="mr")
                nc.vector.tensor_tensor(
                    minres[:, :F], inp[:, cbase : cbase + F], na_bc, Alu.min
                )
                psum2 = ps2pool.tile([128, CHUNK], f32, tag="p2")
                for s0 in range(0, F, 512):
                    Fs = min(512, F - s0)
                    nc.tensor.matmul(
                        psum2[:, s0 : s0 + Fs],
                        ident,
                        d_sl[:, s0 : s0 + Fs],
                        start=True,
                        stop=False,
                    )
                    nc.tensor.matmul(
                        psum2[:, s0 : s0 + Fs],
                        w2neg,
                        minres[:, s0 : s0 + Fs],
                        start=False,
                        stop=True,
                    )
                t2 = t2pool.tile([128, CHUNK], bf, tag="t2")
                r = DVETAIL if F > 512 else 0
                nc.scalar.activation(t2[:, : F - r], psum2[:, : F - r], Act.Relu)
                if r:
                    nc.vector.tensor_scalar(
                        t2[:, F - r : F], psum2[:, F - r : F], 0.0, None, Alu.max
                    )
                for sc in range(0, F, 128):
                    M = min(128, F - sc)
                    ptile, pc = log_slot()
                    nc.tensor.matmul(
                        ptile[:M, pc : pc + 2],
                        t2[:, sc : sc + M],
                        w3sep,
                        start=True,
                        stop=True,
                    )
            flush_log()

    raw = nc.to_json_bytes()
    legal = _legalize_sync(raw)
    nc.to_json_bytes = lambda: legal
    return nc


def _legalize_sync(bir_bytes):
    """Split multi-wait sync_info into single-wait EventSemaphore preludes.

    The walrus build in this container encodes at most one sync-wait command
    per instruction for several ISA structs; Tile emits up to ~9 on the tail
    drain. Semantics are preserved: waits execute in order on the same engine
    ahead of the original instruction.
    """
    import json as _json

    bir = _json.loads(bir_bytes)
    for f in bir["functions"]:
        ctr = [0]
        templates = {}
        for blk in f["blocks"]:
            for ins in blk.get("instructions") or []:
                if ins.get("opcode") == "EventSemaphore":
                    templates.setdefault(ins.get("engine"), ins)
        for blk in f["blocks"]:
            insts = blk.get("instructions")
            if not insts:
                continue
            out = []
            for ins in insts:
                si = ins.get("sync_info") or {}
                waits = si.get("on_wait") or []
                keep = 0 if ins.get("opcode") == "TensorTensor" else 1
                if len(waits) > keep:
                    tpl = templates.get(ins.get("engine"))
                    if tpl is not None:
                        moved = waits[: len(waits) - keep]
                        for w in moved:
                            ctr[0] += 1
                            nw = _json.loads(_json.dumps(tpl))
                            nw["name"] = f"escw_{ctr[0]}"
                            nw["sync_info"] = {"on_update": [], "on_wait": [w]}
                            out.append(nw)
                        si["on_wait"] = waits[len(waits) - keep :]
                out.append(ins)
            blk["instructions"] = out
    return _json.dumps(bir).encode()


def _host_prep(latent_z, stats, W1, b1, W2, b2, We1, be1, We2, be2, We3, be3):
    """Node MLP + A/B/D decomposition on host (<1% of total FLOPs)."""
    x = np.concatenate([latent_z, stats], axis=-1).astype(np.float32)
    h = np.maximum(x @ W1 + b1, 0.0)
    emb = (h @ W2 + b2).reshape(B, N, E)
    A = emb @ We1[:E] + be1  # [B, N, H]
    Bm = emb @ We1[E:]  # [B, N, H]
    D = Bm @ We2 + be2  # [B, N, H]
    # node-major transposed: [H, N*B], col = n*B + b
    A_T = np.ascontiguousarray(A.transpose(2, 1, 0).reshape(H, NB))
    B_T = np.ascontiguousarray(Bm.transpose(2, 1, 0).reshape(H, NB))
    D_T = np.ascontiguousarray(D.transpose(2, 1, 0).reshape(H, NB))
    w2neg = np.zeros((128, 128), np.float32)
    w2neg[:H, :H] = -We2
    w2neg[H:, H:] = -We2
    w3sep = np.zeros((128, 2), np.float32)
    w3sep[:H, 0] = We3[:, 0]
    w3sep[H:, 1] = We3[:, 0]
    ident = np.eye(128, dtype=np.float32)
    return A_T, B_T, D_T, w2neg, w3sep, ident, be3


def _shifted(T, sh):
    """[64, NB] -> [64, NB] shifted left by sh cols, zero-padded."""
    out = np.zeros((H, NB), np.float32)
    if sh < NB:
        out[:, : NB - sh] = T[:, sh:]
    return out


def _assembly_indices():
    """Vectorized mapping of logits[p, outcol+g] -> (b, i_loc, j_loc)."""
    if "asm" in _cache:
        return _cache["asm"]
    blocks, ncol = _block_map()
    rows, cols, bs, ilocs, jlocs = [], [], [], [], []
    for ci, m, c0, sc, M, outcol in blocks:
        p = np.arange(M)
        c = c0 + sc + p  # local col within segment
        jl = 16 * m + 1 + c // B
        b = c % B
        for g in (0, 1):
            rows.append(p)
            cols.append(np.full(M, outcol + g))
            bs.append(b)
            ilocs.append(np.full(M, 16 * m + g))
            jlocs.append(jl)
    out = tuple(np.concatenate(a) for a in (rows, cols, bs, ilocs, jlocs))
    _cache["asm"] = out
    return out


def kernel(**inputs):
    from concourse.bass_utils import run_bass_kernel_spmd

    inp = {k: np.asarray(v, np.float32) for k, v in inputs.items()}
    A_T, B_T, D_T, w2neg, w3sep, ident, be3 = _host_prep(**inp)

    wtail = np.concatenate([w2neg, w3sep, ident], axis=1)  # [128, 258]
    in_maps = []
    for k in range(8):
        sh = 2 * k * B
        bsh = _shifted(B_T, sh)
        dsh = _shifted(D_T, sh)
        napk = np.zeros((128, NSEG * B), np.float32)
        for m in range(NSEG):
            top = 16 * m + 2 * k
            bot = top + 1
            napk[:H, B * m : B * m + B] = -A_T[:, top * B : top * B + B]
            if bot < N:
                napk[H:, B * m : B * m + B] = -A_T[:, bot * B : bot * B + B]
        full = np.empty((128, COL_END), np.float32)
        full[:H, COL_B:COL_D] = full[H:, COL_B:COL_D] = bsh
        full[:H, COL_D:COL_NA] = full[H:, COL_D:COL_NA] = dsh
        full[:, COL_NA:COL_W2N] = napk
        full[:, COL_W2N:] = wtail
        in_maps.append({"inp": np.ascontiguousarray(full.astype(BF16))})

    import time as _time

    nc = _cache.get("nc")
    if nc is None:
        nc = _build_nc()
        _cache["nc"] = nc
    t0 = _time.time()
    res = run_bass_kernel_spmd(nc, in_maps, core_ids=list(range(8)))
    globals()["last_results"] = res
    globals()["last_run_s"] = _time.time() - t0

    rows, cols, bs, ilocs, jlocs = _assembly_indices()
    adj = np.zeros((B, N, N), np.float32)
    for k in range(8):
        lg = np.asarray(res.results[k]["logits"], np.float32)
        i = ilocs + 2 * k
        j = jlocs + 2 * k
        valid = (j < N) & (j > i)
        v = lg[rows[valid], cols[valid]] + float(be3[0])
        ii, jj, bb = i[valid], j[valid], bs[valid]
        adj[bb, ii, jj] = v
        adj[bb, jj, ii] = v
    return adj
